# revision 2
# baseline (speedup 1.0000x reference)
"""AverageAttention Trainium2 kernel (fused fp16/fp8 pipeline).

Computes, per batch b (data-parallel across 8 NeuronCores):
    avg      = cumsum(x, axis=seq) / (pos+1)
    inter    = relu(LN(avg) @ w1 + b1)
    avg_out  = inter @ w2 + b2 + avg
    gates    = [x, avg_out] @ wg + bg
    gated    = sigmoid(gates[:, :D]) * x + sigmoid(gates[:, D:]) * avg_out
returns (gated, avg_out), each [B, S, D].

v2 design (vs the bf16 two-phase baseline):
  - Single software-pipelined loop over 16 seq blocks; the gating matmuls of
    block i-1 are interleaved into block i's PE queue so LN/carry bubbles are
    filled and x/avg_out are consumed straight from SBUF (no DRAM re-read).
  - fp8(e4m3) DoubleRow matmuls (0.5 cyc/row, 2 k-chunks per instr) for FFN2
    and the gating projection; FFN1 stays fp16 (LN output has heavy tails that
    fp8 can't represent accurately enough for avg_out's tolerance).
  - Weights pre-scaled by 16 and activations by 8 before e4m3 quantization to
    keep them out of the subnormal range; the 1/128 product scale folds into
    the PSUM-eviction activation for free.
  - An extra DoubleRow pass adds x8 @ (wg - fp8(wg)) on the x half, halving
    the dominant gating quantization error (host-precomputed residual).
  - Everything else runs fp16: same PE/DVE cost as bf16, 8x less rounding.
  - cumsum via fp16 triangular matmul per 128-block + rank-1 carry inject;
    running block-total prefix rides a [1,512] matmul + fp16 DVE adds.
"""

import os
import sys

if "/opt/trn_rl_repo" not in sys.path:
    sys.path.insert(0, "/opt/trn_rl_repo")

# The NEFF executes via the axon-tunneled PJRT backend; a JAX_PLATFORMS=cpu
# pin (used for running references) would hide the NeuronCores.
if os.environ.get("JAX_PLATFORMS") == "cpu":
    os.environ.pop("JAX_PLATFORMS")

from contextlib import ExitStack

import ml_dtypes
import numpy as np

import concourse.bass as bass
import concourse.mybir as mybir
import concourse.tile as tile
from concourse import bacc
from concourse.bass_utils import run_bass_kernel_spmd

B, S, D = 8, 2048, 1024
P = 128
NBLK = S // P            # 16 seq blocks per core
D2 = 2 * D
KC = D // P              # 8 feature chunks of 128
EPS = 1e-6
AS = 8.0                 # activation fp8 pre-scale
WS = 16.0                # weight fp8 pre-scale
PS = AS * WS             # product scale in fp8 PSUMs

FP32 = mybir.dt.float32
FP16 = mybir.dt.float16
FP8 = mybir.dt.float8e4

AF = mybir.ActivationFunctionType
ALU = mybir.AluOpType
DR = mybir.MatmulPerfMode.DoubleRow


def build_program(has_b1: bool, has_b2: bool, has_bg: bool) -> bacc.Bacc:
    nc = bacc.Bacc("TRN2", target_bir_lowering=False, debug=False, num_devices=8)

    x_d = nc.declare_dram_parameter("x16", [S, D], FP16, isOutput=False)
    xT_d = nc.declare_dram_parameter("xT8", [D, S], FP8, isOutput=False)
    w1_d = nc.declare_dram_parameter("w1g", [D, D], FP16, isOutput=False)
    w2_d = nc.declare_dram_parameter("w28", [D, D], FP8, isOutput=False)
    wg_d = nc.declare_dram_parameter("wg8", [D2, D2], FP8, isOutput=False)
    wgr_d = nc.declare_dram_parameter("wgr8", [D, D2], FP8, isOutput=False)
    tri_d = nc.declare_dram_parameter("tri", [P, P], FP16, isOutput=False)
    iden_d = nc.declare_dram_parameter("iden", [P, P], FP16, isOutput=False)
    inv_d = nc.declare_dram_parameter("invpos", [P, NBLK], FP32, isOutput=False)
    if has_b1:
        b1_d = nc.declare_dram_parameter("b1s", [D], FP32, isOutput=False)
    if has_b2:
        b2_d = nc.declare_dram_parameter("b2", [D], FP32, isOutput=False)
    if has_bg:
        bg_d = nc.declare_dram_parameter("bg128", [D2], FP32, isOutput=False)

    gated_d = nc.declare_dram_parameter("gated", [S, D], FP16, isOutput=True)
    aout_d = nc.declare_dram_parameter("avg_out", [S, D], FP16, isOutput=True)

    x_r = x_d[:].rearrange("(n p) d -> p n d", p=P)        # [128, 16, 1024]
    xT_r = xT_d[:].rearrange("(c p) s -> p c s", p=P)      # [128, 8, 2048]
    aout_r = aout_d[:].rearrange("(n p) d -> p n d", p=P)
    gated_r = gated_d[:].rearrange("(n p) d -> p n d", p=P)
    w1_r = w1_d[:].rearrange("(c p) f -> p c f", p=P)      # [128, 8, 1024]
    w2_r = w2_d[:].rearrange("(c p) f -> p c f", p=P)
    wg_r = wg_d[:].rearrange("(c p) j -> p c j", p=P)      # [128, 16, 2048]
    wgr_r = wgr_d[:].rearrange("(c p) j -> p c j", p=P)    # [128, 8, 2048]

    with tile.TileContext(nc) as tc, ExitStack() as ctx:
        const = ctx.enter_context(tc.tile_pool(name="const", bufs=1))

        # persistent transposed-activation stores for the gating lhsT
        aoT8 = const.tile([P, KC, S], FP8)     # 8*avg_out transposed

        cum_ps = ctx.enter_context(tc.tile_pool(name="cum_ps", bufs=2, space="PSUM"))
        mm_ps = ctx.enter_context(tc.tile_pool(name="mm_ps", bufs=3, space="PSUM"))
        tr_ps = ctx.enter_context(tc.tile_pool(name="tr_ps", bufs=1, space="PSUM"))
        tot_ps = ctx.enter_context(tc.tile_pool(name="tot_ps", bufs=1, space="PSUM"))

        xq_p = ctx.enter_context(tc.tile_pool(name="xq", bufs=4))
        avg_p = ctx.enter_context(tc.tile_pool(name="avg", bufs=3))
        z_p = ctx.enter_context(tc.tile_pool(name="z", bufs=2))
        ln_p = ctx.enter_context(tc.tile_pool(name="lnT", bufs=2))
        int_p = ctx.enter_context(tc.tile_pool(name="intT", bufs=2))
        ao_p = ctx.enter_context(tc.tile_pool(name="ao", bufs=3))
        sig_p = ctx.enter_context(tc.tile_pool(name="sig", bufs=2))
        g_p = ctx.enter_context(tc.tile_pool(name="g", bufs=3))
        stat_p = ctx.enter_context(tc.tile_pool(name="stat", bufs=6))
        incl_p = ctx.enter_context(tc.tile_pool(name="incl", bufs=2))

        # ---- constants & weights -------------------------------------------
        iden_sb = const.tile([P, P], FP16)
        nc.sync.dma_start(out=iden_sb, in_=iden_d[:])
        tri_sb = const.tile([P, P], FP16)
        nc.sync.dma_start(out=tri_sb, in_=tri_d[:])
        inv_sb = const.tile([P, NBLK], FP32)
        nc.sync.dma_start(out=inv_sb, in_=inv_d[:])
        ones_row = tri_sb[0:1, :]              # [1, 128] of ones
        ones_col = tri_sb[:, P - 1 : P]        # [128, 1] of ones
        # int32 seed constant for the DVE fast-inverse-sqrt (keeps Sqrt off
        # ScalarE so the kernel needs no mid-stream LoadActFuncSet)
        magic_sb = const.tile([P, 1], mybir.dt.int32)
        nc.vector.memset(magic_sb, 0x5F3759DF)
        if has_b1:
            b1t_sb = const.tile([P, KC], FP32)
            nc.sync.dma_start(out=b1t_sb, in_=b1_d[:].rearrange("(c p) -> p c", p=P))
        if has_b2:
            b2r_sb = const.tile([P, D], FP32)
            nc.sync.dma_start(out=b2r_sb, in_=b2_d[None, :].to_broadcast([P, D]))
        if has_bg:
            bgr_sb = const.tile([P, D2], FP32)
            nc.sync.dma_start(out=bgr_sb, in_=bg_d[None, :].to_broadcast([P, D2]))

        x_tiles = {}

        def issue_x(b):
            if b >= NBLK:
                return
            t = xq_p.tile([P, D], FP16, tag="xq")
            nc.sync.dma_start(out=t, in_=x_r[:, b, :])
            x_tiles[b] = t

        issue_x(0)
        issue_x(1)

        w1_sb = const.tile([P, KC, D], FP16)
        nc.gpsimd.dma_start(out=w1_sb, in_=w1_r)
        xT8_sb = const.tile([P, KC, S], FP8)
        nc.gpsimd.dma_start(out=xT8_sb, in_=xT_r)
        wg_sb = const.tile([P, 2 * KC, D2], FP8)
        # stream wg in j-column strips ordered by first gating use
        for j in range(4):
            nc.gpsimd.dma_start(
                out=wg_sb[:, :, j * 512 : (j + 1) * 512],
                in_=wg_r[:, :, j * 512 : (j + 1) * 512],
            )
            if j == 0:
                w2_sb = const.tile([P, KC, D], FP8)
                nc.gpsimd.dma_start(out=w2_sb, in_=w2_r)
        wgr_sb = const.tile([P, KC, D2], FP8)
        nc.gpsimd.dma_start(out=wgr_sb, in_=wgr_r)

        def transpose_blk(src_ap, ps_tag):
            """Transpose a [128, 1024] fp16 block via 8 PE transposes into one
            [128, 1024] fp16 PSUM tile (2KB = one bank)."""
            ptr = tr_ps.tile([P, D], FP16, tag=ps_tag)
            for j in range(KC):
                nc.tensor.transpose(
                    ptr[:, j * P : (j + 1) * P],
                    src_ap[:, j * P : (j + 1) * P],
                    iden_sb,
                )
            return ptr

        state = {}

        def cumsum_ln_ffn1(i):
            """cumsum -> avg -> LN -> zT -> FFN1 for block i."""
            x_b = x_tiles.pop(i)
            issue_x(i + 2)
            prev_incl = state.get("incl")

            # -- in-block cumsum (+carry inject) in 2 PSUM banks -------------
            cps = []
            for c in range(2):
                cs = slice(c * 512, (c + 1) * 512)
                ps = cum_ps.tile([P, 512], FP32, tag="cum")
                nc.tensor.matmul(
                    ps, lhsT=tri_sb[:], rhs=x_b[:, cs],
                    start=True, stop=(i == 0),
                )
                if i > 0:
                    nc.tensor.matmul(
                        ps, lhsT=ones_row, rhs=prev_incl[0:1, cs],
                        start=False, stop=True,
                    )
                cps.append(ps)

            # -- running prefix of block totals (fp16 [1, 1024]) -------------
            if i < NBLK - 1:
                cur_incl = incl_p.tile([1, D], FP16, tag="incl")
                for c in range(2):
                    cs = slice(c * 512, (c + 1) * 512)
                    pst = tot_ps.tile([1, 512], FP32, tag="tot")
                    nc.tensor.matmul(
                        pst, lhsT=ones_col, rhs=x_b[:, cs],
                        start=True, stop=True,
                    )
                    if i == 0:
                        nc.gpsimd.tensor_copy(out=cur_incl[0:1, cs], in_=pst)
                    else:
                        nc.vector.scalar_tensor_tensor(
                            out=cur_incl[0:1, cs], in0=pst, scalar=0.0,
                            in1=prev_incl[0:1, cs], op0=ALU.bypass, op1=ALU.add,
                        )
                state["incl"] = cur_incl

            # -- scale-evict to cumulative average (fp16) --------------------
            avg_b = avg_p.tile([P, D], FP16, tag="avg")
            for c in range(2):
                cs = slice(c * 512, (c + 1) * 512)
                nc.scalar.mul(out=avg_b[:, cs], in_=cps[c], mul=inv_sb[:, i : i + 1])
            if has_b2:
                nc.gpsimd.tensor_add(out=avg_b, in0=avg_b, in1=b2r_sb)

            # -- LayerNorm stats + fast rsqrt (all DVE) ----------------------
            st = stat_p.tile([P, 2, 6], FP32, tag="st")
            for g in range(2):
                nc.vector.bn_stats(out=st[:, g, :], in_=avg_b[:, g * 512 : (g + 1) * 512])
            mv = stat_p.tile([P, 2], FP32, tag="mv")
            nc.vector.bn_aggr(out=mv, in_=st)
            y = stat_p.tile([P, 1], FP32, tag="y")
            nc.vector.tensor_scalar(
                out=y, in0=mv[:, 1:2], scalar1=EPS, scalar2=None, op0=ALU.add
            )
            r0b = stat_p.tile([P, 1], mybir.dt.int32, tag="r0b")
            nc.vector.tensor_scalar(
                out=r0b, in0=y[:].bitcast(mybir.dt.int32), scalar1=1,
                scalar2=None, op0=ALU.logical_shift_right,
            )
            nc.vector.tensor_tensor(out=r0b, in0=magic_sb, in1=r0b, op=ALU.subtract)
            rstd = r0b[:].bitcast(FP32)
            t = stat_p.tile([P, 1], FP32, tag="t")
            for _ in range(3):
                nc.vector.tensor_tensor(out=t, in0=rstd, in1=rstd, op=ALU.mult)
                nc.vector.tensor_tensor(out=t, in0=t, in1=y, op=ALU.mult)
                nc.vector.tensor_scalar(
                    out=t, in0=t, scalar1=-0.5, scalar2=1.5,
                    op0=ALU.mult, op1=ALU.add,
                )
                nc.vector.tensor_tensor(out=rstd, in0=rstd, in1=t, op=ALU.mult)
            z_b = z_p.tile([P, D], FP16, tag="z")
            nc.vector.tensor_scalar(
                out=z_b, in0=avg_b, scalar1=mv[:, 0:1], scalar2=rstd,
                op0=ALU.subtract, op1=ALU.mult,
            )

            # -- transpose z; evict to fp16 lnT on GPSIMD --------------------
            zps = transpose_blk(z_b, "trz")
            lnT_b = ln_p.tile([P, KC, P], FP16, tag="lnT")
            nc.gpsimd.tensor_copy(
                out=lnT_b, in_=zps[:].rearrange("p (c s) -> p c s", c=KC)
            )

            # -- FFN1 (fp16): interT[fc, s] = 8*relu(w1'.T @ lnT + b1') ------
            fps = []
            for h in range(2):
                ps = mm_ps.tile([P, 512], FP32, tag="mm")
                for fc in range(4 * h, 4 * h + 4):
                    col = slice((fc - 4 * h) * P, (fc - 4 * h + 1) * P)
                    for k in range(KC):
                        nc.tensor.matmul(
                            ps[:, col],
                            lhsT=w1_sb[:, k, fc * P : (fc + 1) * P],
                            rhs=lnT_b[:, k, :],
                            start=(k == 0), stop=(k == KC - 1),
                        )
                fps.append(ps)
            intT_b = int_p.tile([P, KC, P], FP8, tag="intT")
            for h in range(2):
                if has_b1:
                    for fc in range(4 * h, 4 * h + 4):
                        col = slice((fc - 4 * h) * P, (fc - 4 * h + 1) * P)
                        nc.scalar.activation(
                            out=intT_b[:, fc, :], in_=fps[h][:, col],
                            func=AF.Relu, bias=b1t_sb[:, fc : fc + 1], scale=AS,
                        )
                else:
                    nc.scalar.activation(
                        out=intT_b[:, 4 * h : 4 * h + 4, :],
                        in_=fps[h][:].rearrange("p (c s) -> p c s", c=4),
                        func=AF.Relu, scale=AS,
                    )
            state[("lnT", i)] = lnT_b
            state[("intT", i)] = intT_b
            state[("avg", i)] = avg_b
            state[("x", i)] = x_b

        def ffn2(i):
            """FFN2 (fp8 DoubleRow) + residual -> ao fp16; transpose to aoT8."""
            intT_b = state.pop(("intT", i))
            avg_b = state.pop(("avg", i))
            ao_b = ao_p.tile([P, D], FP16, tag="ao")
            for c in range(2):
                cs = slice(c * 512, (c + 1) * 512)
                ps = mm_ps.tile([P, 512], FP32, tag="mm")
                for f in range(KC // 2):
                    nc.tensor.matmul(
                        ps,
                        lhsT=intT_b[:, 2 * f : 2 * f + 2, :],
                        rhs=w2_sb[:, 2 * f : 2 * f + 2, cs],
                        start=(f == 0), stop=(f == KC // 2 - 1),
                        perf_mode=DR,
                    )
                nc.vector.scalar_tensor_tensor(
                    out=ao_b[:, cs], in0=ps, scalar=1.0 / PS,
                    in1=avg_b[:, cs], op0=ALU.mult, op1=ALU.add,
                )
            nc.sync.dma_start(out=aout_r[:, i, :], in_=ao_b)
            aps = transpose_blk(ao_b, "trao")
            nc.gpsimd.tensor_scalar(
                out=aoT8[:, :, i * P : (i + 1) * P],
                in0=aps[:].rearrange("p (c s) -> p c s", c=KC),
                scalar1=AS, scalar2=None, op0=ALU.mult,
            )
            state[("ao", i)] = ao_b

        def gate(i, half):
            """Gating for block i, j-half `half` (in: 0, fg: 1): two 512-wide
            PSUMs, sigmoid evict, and on the fg half the final combine+DMA."""
            icols = slice(i * P, (i + 1) * P)
            if half == 0:
                sig_b = sig_p.tile([P, 2, D], FP16, tag="sig")
                state[("sig", i)] = sig_b
            else:
                sig_b = state[("sig", i)]
            for dh in range(2):
                jcol = slice(half * D + dh * 512, half * D + (dh + 1) * 512)
                ps = mm_ps.tile([P, 512], FP32, tag="mm")
                for k in range(KC):  # 8 pairs over [x; ao] features
                    wsrc = xT8_sb if k < KC // 2 else aoT8
                    kk = 2 * k if k < KC // 2 else 2 * (k - KC // 2)
                    nc.tensor.matmul(
                        ps,
                        lhsT=wsrc[:, kk : kk + 2, icols],
                        rhs=wg_sb[:, 2 * k : 2 * k + 2, jcol],
                        start=(k == 0), stop=False,
                        perf_mode=DR,
                    )
                for k in range(KC // 2):  # x-side wg residual compensation
                    nc.tensor.matmul(
                        ps,
                        lhsT=xT8_sb[:, 2 * k : 2 * k + 2, icols],
                        rhs=wgr_sb[:, 2 * k : 2 * k + 2, jcol],
                        start=False, stop=(k == KC // 2 - 1),
                        perf_mode=DR,
                    )
                if has_bg:
                    gsb = stat_p.tile([P, 512], FP32, tag="gbias")
                    nc.vector.scalar_tensor_tensor(
                        out=gsb, in0=ps, scalar=0.0, in1=bgr_sb[:, jcol],
                        op0=ALU.bypass, op1=ALU.add,
                    )
                    nc.scalar.activation(
                        out=sig_b[:, half, dh * 512 : (dh + 1) * 512],
                        in_=gsb, func=AF.Sigmoid, scale=1.0 / PS,
                    )
                else:
                    nc.scalar.activation(
                        out=sig_b[:, half, dh * 512 : (dh + 1) * 512],
                        in_=ps, func=AF.Sigmoid, scale=1.0 / PS,
                    )
            if half == 1:
                x_b = state.pop(("x", i))
                ao_b = state.pop(("ao", i))
                sig_b = state.pop(("sig", i))
                m1 = g_p.tile([P, D], FP16, tag="m1")
                nc.vector.scalar_tensor_tensor(
                    out=m1, in0=x_b, scalar=0.0, in1=sig_b[:, 0, :],
                    op0=ALU.bypass, op1=ALU.mult,
                )
                m2 = g_p.tile([P, D], FP16, tag="m2")
                nc.vector.scalar_tensor_tensor(
                    out=m2, in0=ao_b, scalar=0.0, in1=sig_b[:, 1, :],
                    op0=ALU.bypass, op1=ALU.mult,
                )
                gt = g_p.tile([P, D], FP16, tag="gt")
                nc.vector.scalar_tensor_tensor(
                    out=gt, in0=m1, scalar=0.0, in1=m2,
                    op0=ALU.bypass, op1=ALU.add,
                )
                nc.sync.dma_start(out=gated_r[:, i, :], in_=gt)

        # ---- software-pipelined main loop ----------------------------------
        for i in range(NBLK):
            cumsum_ln_ffn1(i)
            if i >= 1:
                gate(i - 1, 0)
            ffn2(i)
            if i >= 1:
                gate(i - 1, 1)
        gate(NBLK - 1, 0)
        gate(NBLK - 1, 1)

    nc.compile()
    return nc


def host_inputs(x, w1, b1, w2, b2, ln_g, ln_b, wg, bg):
    """Fold LN affine params into w1/b1, pre-scale + quantize fp8 operands,
    transpose x on host, precompute constants."""
    x = np.asarray(x, np.float32)
    w1 = np.asarray(w1, np.float32)
    w2 = np.asarray(w2, np.float32)
    wg = np.asarray(wg, np.float32)
    ln_g = np.asarray(ln_g, np.float32)
    ln_b = np.asarray(ln_b, np.float32)
    b1 = np.asarray(b1, np.float32)

    E4 = ml_dtypes.float8_e4m3
    w1g = (ln_g[:, None] * w1).astype(np.float16)
    b1p = (b1 + ln_b @ w1).astype(np.float32)
    wg8 = (WS * wg).astype(E4)
    wgr8 = (WS * wg[:D] - wg8[:D].astype(np.float32)).astype(E4)
    tri = np.triu(np.ones((P, P), np.float16))
    iden = np.eye(P, dtype=np.float16)
    pos = np.arange(S, dtype=np.float64).reshape(NBLK, P).T  # [P, NBLK]
    invpos = (1.0 / (pos + 1.0)).astype(np.float32)

    base = {
        "x16": None,   # per-core
        "xT8": None,   # per-core
        "w1g": w1g,
        "w28": (WS * w2).astype(E4),
        "wg8": wg8,
        "wgr8": wgr8,
        "tri": tri,
        "iden": iden,
        "invpos": invpos,
    }
    has_b1 = bool(np.any(b1p))
    has_b2 = bool(np.any(b2))
    has_bg = bool(np.any(bg))
    if has_b1:
        base["b1s"] = AS * b1p
    if has_b2:
        base["b2"] = np.asarray(b2, np.float32)
    if has_bg:
        base["bg128"] = PS * np.asarray(bg, np.float32)
    return base, has_b1, has_b2, has_bg


_prog_cache = {}


def kernel(x, w1, b1, w2, b2, ln_g, ln_b, wg, bg):
    x = np.asarray(x, np.float32)
    assert x.shape == (B, S, D), x.shape
    base, has_b1, has_b2, has_bg = host_inputs(
        x, w1, b1, w2, b2, ln_g, ln_b, wg, bg
    )

    key = (has_b1, has_b2, has_bg)
    if key not in _prog_cache:
        _prog_cache[key] = build_program(has_b1, has_b2, has_bg)
    nc = _prog_cache[key]

    E4 = ml_dtypes.float8_e4m3
    x16 = x.astype(np.float16)
    in_maps = []
    for core in range(B):
        m = dict(base)
        m["x16"] = np.ascontiguousarray(x16[core])
        m["xT8"] = np.ascontiguousarray(
            (AS * x16[core].astype(np.float32).T).astype(E4)
        )
        in_maps.append(m)

    res = run_bass_kernel_spmd(nc, in_maps, core_ids=list(range(B)))
    gated = np.stack(
        [res.results[c]["gated"].astype(np.float32) for c in range(B)]
    )
    avg_out = np.stack(
        [res.results[c]["avg_out"].astype(np.float32) for c in range(B)]
    )
    return gated, avg_out
